# revision 66
# baseline (speedup 1.0000x reference)
"""Distributed Trainium2 kernel for the dense-graph GNN layer.

Math: with xn = x/||x|| (rows), G = xn@xn.T, d = rsqrt(G@1),
out = (diag(d) G diag(d) x) W.  The N x N Gram matrix is never needed:
  G @ 1        = xn @ t,            t = colsum(xn)            [D]
  diag(d) G diag(d) x = f * (x @ z),  z = x.T @ diag(f) @ x   [D, D]
  f_i = d_i / ||x_i||   (combines both scalings; z is symmetric)
  out = f * (x @ (z @ W))
Each core processes its 1024-row shard; the only cross-core traffic is
(1) a reduction of the [D] colsum partial and (2) a reduction of the
[D, D] (z @ W) partial.  Both reductions run as recursive-doubling
exchanges over direct peer-to-peer SBUF remote DMAs (XOR partners 1, 2,
4), which avoids both the ~70us ncfw collective bringup and the
descriptor flood of a full mesh.  A dangling 1-byte AllGather marks the
NEFF as collective so the runtime gang-launches the 8 cores (without
it, dispatch is staggered by milliseconds).
"""

import os
import sys

import numpy as np

for _p in ("/opt/trn_rl_repo", "/root/.axon_site/_ro/trn_rl_repo"):
    if os.path.isdir(_p) and _p not in sys.path:
        sys.path.insert(0, _p)

import concourse.bacc as bacc
import concourse.mybir as mybir
import concourse.tile as tile
import concourse.masks as masks
from concourse import bass_utils

R = 8                 # cores
N, D = 8192, 256
NL = N // R           # 1024 rows per core
P = 128
T = NL // P           # 8 row tiles per core
F32 = mybir.dt.float32
BF16 = mybir.dt.bfloat16
AF = mybir.ActivationFunctionType
ALU = mybir.AluOpType

TSLOT = 8             # t exchange payload width (f32 cols) = 32 B/partition
ZSLOT = 2 * D         # zw exchange payload width (bf16 cols) = 1 KB/partition
HOPS = (4, 2, 1)      # recursive-doubling XOR distances (cross-die first)

_cache = {}


def _rdests(dq, big):
    """8-slot dest list.  Every real slot ships a full 128-partition copy
    on its two lanes (64 descriptors/lane — invariant), so slots trade
    replication wire against dummy-lane trickle.  Small payloads (t) use
    all-same-dest slots wherever allowed: no dummy descriptors, and the
    replicated wire is negligible.  The 1 KB/partition zw payload keeps
    4 copies (halves the wire).  Cross-die dests (dq=4) may only occupy
    the D2D-capable slots 4-7.  Receiver sem increments: 2 per real slot."""
    if dq == 4:
        return [None, None, None, None, (0, dq), (0, dq), (0, dq), (0, dq)]
    if big:
        return [(0, dq)] * 4 + [None] * 4
    return [(0, dq)] * 8


def _rinc(dq, big):
    return 8 if (dq == 4 or big) else 16


def _program(tc, x, W, out):
    nc = tc.nc
    # Per-hop arrival sems (a shared counter would be ambiguous: a fast
    # partner's hop-2 arrival must not satisfy a hop-1 wait).
    rsem_t = [nc.alloc_semaphore(f"rsem_t{k}") for k in range(3)]
    rsem_z = [nc.alloc_semaphore(f"rsem_z{k}") for k in range(3)]
    # Local (send-drained) sems, one per SWDGE queue.
    lsem = [nc.alloc_semaphore(f"lsem_q{q}") for q in range(4)]
    with (
        tc.tile_pool(name="persist", bufs=1) as pp,
        tc.tile_pool(name="work", bufs=3) as wp,
        tc.tile_pool(name="psum", bufs=1, space="PSUM") as psp,
        tc.tile_pool(name="psumw", bufs=4, space="PSUM") as psw,
        tc.tile_pool(name="dram", bufs=1, space="DRAM") as dp,
    ):
        # Dangling 1-byte AllGather: marks the NEFF as collective so the
        # runtime gang-launches the 8 cores; nothing waits on it.
        cc_in = dp.tile([1, 1], F32)
        cc_out = dp.tile([R, 1], F32)
        nc.gpsimd.collective_compute(
            "AllGather", ALU.bypass, replica_groups=[list(range(R))],
            ins=[cc_in.opt()], outs=[cc_out.opt()],
        )

        ident = pp.tile([P, P], F32)
        masks.make_identity(nc, ident[:])
        x_all = pp.tile([P, T * D], F32)      # row tile i at [:, i*D:(i+1)*D]
        xb_all = pp.tile([P, T * D], BF16)    # bf16 copy of x
        g_all = pp.tile([P, T * D], BF16)     # f * x (bf16)
        xT_all = pp.tile([P, 2 * NL], BF16)   # x.T chunk c at [:, c*NL + i*P]
        W_sb = pp.tile([P, 2 * D], F32)       # W k-chunk kc at [:, kc*D]
        Wb_sb = pp.tile([P, 2 * D], BF16)

        ss = pp.tile([P, T], F32)
        invn = pp.tile([P, T], F32)
        nrm = pp.tile([P, T], F32)
        stl = pp.tile([P, T], F32)
        s_t = pp.tile([P, T], F32)
        sq_s = pp.tile([P, T], F32)
        dd = pp.tile([P, T], F32)
        f_t = pp.tile([P, T], F32)


        # Exchange buffers.  t_col / zw_loc accumulate in place; thr/zhr
        # receive the partner's running sum each hop.
        t_col = pp.tile([P, TSLOT], F32)      # my colsum partial, cols 0-1
        thr = [pp.tile([P, TSLOT], F32, name=f"thr{k}") for k in range(3)]
        ta = [t_col] + [pp.tile([P, TSLOT], F32, name=f"ta{k}") for k in range(3)]
        zw_loc = pp.tile([P, ZSLOT], BF16)    # my (z @ W) partial
        zhr = [pp.tile([P, ZSLOT], BF16, name=f"zhr{k}") for k in range(3)]
        za = [zw_loc] + [pp.tile([P, ZSLOT], BF16, name=f"za{k}") for k in range(3)]

        nc.gpsimd.memset(t_col[:], 0.0)

        # Hop k of both reductions lives on SWDGE queue k: a queue never
        # hosts two consecutive hops, so one hop's background dummy-lane
        # trickle cannot delay the next hop's descriptors.  The t-hop
        # preps are emitted here (desc-gen only, reads deferred).
        for k in range(3):
            nc.gpsimd.remote_dma_broadcast(
                thr[k][:], ta[k][:], rsem_t[k], lsem[k],
                rdests=_rdests(HOPS[k], False), queue_num=k,
            )

        for kc in range(2):
            nc.sync.dma_start(W_sb[:, kc * D:(kc + 1) * D], W[kc * P:(kc + 1) * P, :])
        nc.vector.tensor_copy(Wb_sb[:], W_sb[:])

        # ---- phase A: load shard, row norms, colsum(xn) partial ----
        from concourse.bass_types import AP as _AP
        for i in range(T):
            xs = x_all[:, i * D:(i + 1) * D]
            nc.sync.dma_start(xs, x[i * P:(i + 1) * P, :])
            scr = wp.tile([P, D], F32, tag="scr", name=f"scr{i}")
            nc.scalar.activation(scr[:], xs, AF.Square, accum_out=ss[:, i:i + 1])
            nc.vector.tensor_copy(xb_all[:, i * D:(i + 1) * D], xs)
        nc.scalar.activation(nrm[:], ss[:], AF.Sqrt)
        nc.vector.reciprocal(invn[:], nrm[:])
        invn_b = pp.tile([P, T], BF16)
        nc.vector.tensor_copy(invn_b[:], invn[:])

        # t partial in column layout: t_col[p, c] = sum_i (x_i chunk c)^T invn_i
        # (bf16 operands: one-pass matmuls, ~3x faster than fp32)
        psum_tc = psw.tile([P, D], F32, tag="pw", name="psum_tc")
        for c in range(2):
            for i in range(T):
                nc.tensor.matmul(
                    psum_tc[:, c:c + 1],
                    lhsT=xb_all[:, i * D + c * P:i * D + (c + 1) * P],
                    rhs=invn_b[:, i:i + 1],
                    start=(i == 0), stop=(i == T - 1),
                )
        nc.vector.tensor_copy(t_col[:, 0:2], psum_tc[:, 0:2])

        # x.T via PE transposes (independent work that overlaps the exchange)
        for i in range(T):
            for c in range(2):
                pt = psw.tile([P, P], F32, tag="pw", name=f"pt{i}_{c}")
                nc.tensor.transpose(
                    pt[:], x_all[:, i * D + c * P: i * D + (c + 1) * P], ident[:]
                )
                nc.vector.tensor_copy(xT_all[:, c * NL + i * P: c * NL + (i + 1) * P], pt[:])

        # y = x @ W per tile, also during the exchange window.  The zw
        # partial is then g^T @ y directly — no zT staging roundtrip.
        y_all = pp.tile([P, T * D], BF16)
        for i in range(T):
            py = psw.tile([P, D], F32, tag="pw", name=f"py{i}")
            for c in range(2):
                nc.tensor.matmul(
                    py[:], lhsT=xT_all[:, c * NL + i * P:c * NL + (i + 1) * P],
                    rhs=Wb_sb[:, c * D:(c + 1) * D],
                    start=(c == 0), stop=(c == 1),
                )
            nc.vector.tensor_copy(y_all[:, i * D:(i + 1) * D], py[:])

        # ---- t recursive doubling.  No entry barrier is needed:
        # target_bir_lowering is off so there is no per-kernel sem clear —
        # sems are zeroed at NEFF load and remote increments persist even if
        # a peer has not started executing yet.  Each trigger's
        # signals_writable gives it a WAW edge after the t_col producer (the
        # preps predate the producer, so the deferred-RAW edge never forms),
        # orders the hop's consumer add after it, and pins the next hop's
        # prep behind it in the queue-0 FIFO.
        add_t = []
        for k in range(3):
            # signals: WAW after the ta[k] producer, before the thr[k]
            # consumer, and pinning queue-k's zw prep (writes zhr[k])
            # behind this trigger in the FIFO.
            nc.gpsimd.trigger_dma(
                count=None, queue_num=k,
                signals_writable=(ta[k][:], thr[k][:], zhr[k][:]),
            )
            # Double-buffered: the sum lands in a fresh tile, so the hop's
            # in-flight send never races the accumulation (no local-sem wait).
            add_t.append(nc.vector.tensor_add(ta[k + 1][:], ta[k][:], thr[k][:]))

        # zw-hop send descriptors, queue k (pinned behind t-trigger k).
        for k, dq in enumerate(HOPS):
            nc.gpsimd.remote_dma_broadcast(
                zhr[k][:], za[k][:], rsem_z[k], lsem[k],
                rdests=_rdests(dq, True), queue_num=k,
            )

        # ---- phase B: degrees, f, g = f*x, zT partial, zw partial ----
        # Per-row dot products s = x . t directly on the PE using the
        # already-transposed x: psum_s[:, i] = sum_c xT(c,i)^T @ t_col_b[c]
        # — no 128-partition broadcast of t and no big multiply+reduce.
        tcol_b = pp.tile([P, 2], BF16)
        nc.vector.tensor_copy(tcol_b[:], ta[3][:, 0:2])
        psum_s = psw.tile([P, D], F32, tag="pw", name="psum_s")
        for i in range(T):
            for c in range(2):
                nc.tensor.matmul(
                    psum_s[:, i:i + 1],
                    lhsT=xT_all[:, c * NL + i * P:c * NL + (i + 1) * P],
                    rhs=tcol_b[:, c:c + 1],
                    start=(c == 0), stop=(c == 1),
                )
        nc.vector.tensor_copy(stl[:], psum_s[:, 0:T])
        nc.vector.tensor_mul(s_t[:], stl[:], invn[:])       # s = rowsum * invn
        nc.scalar.activation(sq_s[:], s_t[:], AF.Sqrt)
        nc.vector.reciprocal(dd[:], sq_s[:])                # d = rsqrt(s)
        nc.vector.tensor_mul(f_t[:], dd[:], invn[:])        # f = d * invn
        # Split the per-row scaling g = f*x across ACT and DVE so neither
        # engine serializes all eight tiles.
        for i in range(T):
            gs = g_all[:, i * D:(i + 1) * D]
            xs = x_all[:, i * D:(i + 1) * D]
            if i % 2 == 0:
                nc.scalar.mul(gs, xs, f_t[:, i:i + 1])
            else:
                nc.vector.tensor_scalar_mul(gs, xs, f_t[:, i:i + 1])

        # zw partial = (g^T y) chunk m, accumulated over the row tiles.
        psum_zw0 = psp.tile([P, D], F32, name="pzw0")
        psum_zw1 = psp.tile([P, D], F32, name="pzw1")
        for i in range(T):
            for m, pz in ((0, psum_zw0), (1, psum_zw1)):
                nc.tensor.matmul(
                    pz[:], lhsT=g_all[:, i * D + m * P: i * D + (m + 1) * P],
                    rhs=y_all[:, i * D:(i + 1) * D],
                    start=(i == 0), stop=(i == T - 1),
                )
        for m, pz in ((0, psum_zw0), (1, psum_zw1)):
            nc.vector.tensor_copy(zw_loc[:, m * D:(m + 1) * D], pz[:])

        # ---- zw recursive doubling (hops on queues 1-3) ----
        add_z = []
        for k in range(3):
            nc.gpsimd.trigger_dma(count=None, queue_num=k,
                                  signals_writable=(za[k][:], zhr[k][:]))
            add_z.append(nc.vector.tensor_add(za[k + 1][:], za[k][:], zhr[k][:]))

        # ---- phase C: out = f * (x @ zw) ----
        for i in range(T):
            po = psw.tile([P, D], F32, tag="pw", name=f"po{i}")
            for ka in range(2):
                nc.tensor.matmul(
                    po[:], lhsT=xT_all[:, ka * NL + i * P: ka * NL + (i + 1) * P],
                    rhs=za[3][:, ka * D:(ka + 1) * D],
                    start=(ka == 0), stop=(ka == 1),
                )
            o_sb = wp.tile([P, D], F32, tag="osb", name=f"osb{i}")
            nc.scalar.mul(o_sb[:], po[:], f_t[:, i:i + 1])
            nc.sync.dma_start(out[i * P:(i + 1) * P, :], o_sb[:])

    return {"add_t": add_t, "add_z": add_z,
            "rsem_t": rsem_t, "rsem_z": rsem_z, "lsem": lsem}


def _build():
    nc = bacc.Bacc("TRN2", target_bir_lowering=False, debug=False, num_devices=R,
                   num_swdge_queues=4)
    x = nc.dram_tensor("x", [NL, D], F32, kind="ExternalInput")
    W = nc.dram_tensor("W", [D, D], F32, kind="ExternalInput")
    out = nc.dram_tensor("out", [NL, D], F32, kind="ExternalOutput")
    with tile.TileContext(nc) as tc:
        h = _program(tc, x.ap() if hasattr(x, "ap") else x, W.ap() if hasattr(W, "ap") else W, out.ap() if hasattr(out, "ap") else out)
    # Attach the cross-core waits after scheduling (the schedule-time
    # single-core sim cannot model peer sem increments, and added waits
    # only delay — they cannot invalidate the schedule).  Each hop's add
    # waits for the partner's payload (+2 on the hop's remote sem) and for
    # this core's own send of the hop to drain (+16 on the queue's local
    # sem) before overwriting the send buffer.  compile() splits
    # multi-wait instructions into event semaphores automatically.
    for k in range(3):
        h["add_t"][k].wait_op(h["rsem_t"][k], _rinc(HOPS[k], False), "sem-ge", check=False)
        h["add_z"][k].wait_op(h["rsem_z"][k], _rinc(HOPS[k], True), "sem-ge", check=False)
    nc.finalize()
    return nc


def _run(inputs, trace=False):
    if "nc" not in _cache:
        _cache["nc"] = _build()
    nc = _cache["nc"]
    x = np.ascontiguousarray(inputs["x"], dtype=np.float32)
    W = np.ascontiguousarray(inputs["W"], dtype=np.float32)
    in_maps = [{"x": x[r * NL:(r + 1) * NL], "W": W} for r in range(R)]
    res = bass_utils.run_bass_kernel_spmd(
        nc, in_maps, core_ids=list(range(R)), trace=trace,
    )
    out = np.concatenate([res.results[r]["out"] for r in range(R)], axis=0)
    return out, res


def kernel(**inputs) -> np.ndarray:
    out, _ = _run(inputs, trace=False)
    return out


# revision 67
# speedup vs baseline: 1.0834x; 1.0834x over previous
"""Distributed Trainium2 kernel for the dense-graph GNN layer.

Math: with xn = x/||x|| (rows), G = xn@xn.T, d = rsqrt(G@1),
out = (diag(d) G diag(d) x) W.  The N x N Gram matrix is never needed:
  G @ 1        = xn @ t,            t = colsum(xn)            [D]
  diag(d) G diag(d) x = f * (x @ z),  z = x.T @ diag(f) @ x   [D, D]
  f_i = d_i / ||x_i||   (combines both scalings; z is symmetric)
  out = f * (x @ (z @ W))
Each core processes its 1024-row shard; the only cross-core traffic is
(1) a reduction of the [D] colsum partial and (2) a reduction of the
[D, D] (z @ W) partial.  Both reductions run as recursive-doubling
exchanges over direct peer-to-peer SBUF remote DMAs (XOR partners 1, 2,
4), which avoids both the ~70us ncfw collective bringup and the
descriptor flood of a full mesh.  A dangling 1-byte AllGather marks the
NEFF as collective so the runtime gang-launches the 8 cores (without
it, dispatch is staggered by milliseconds).
"""

import os
import sys

import numpy as np

for _p in ("/opt/trn_rl_repo", "/root/.axon_site/_ro/trn_rl_repo"):
    if os.path.isdir(_p) and _p not in sys.path:
        sys.path.insert(0, _p)

import concourse.bacc as bacc
import concourse.mybir as mybir
import concourse.tile as tile
import concourse.masks as masks
from concourse import bass_utils

R = 8                 # cores
N, D = 8192, 256
NL = N // R           # 1024 rows per core
P = 128
T = NL // P           # 8 row tiles per core
F32 = mybir.dt.float32
BF16 = mybir.dt.bfloat16
AF = mybir.ActivationFunctionType
ALU = mybir.AluOpType

TSLOT = 8             # t exchange payload width (f32 cols) = 32 B/partition
ZSLOT = 2 * D         # zw exchange payload width (bf16 cols) = 1 KB/partition
HOPS = (4, 2, 1)      # recursive-doubling XOR distances (cross-die first)

_cache = {}


def _rdests(dq, big):
    """8-slot dest list.  Every real slot ships a full 128-partition copy
    on its two lanes (64 descriptors/lane — invariant), so slots trade
    replication wire against dummy-lane trickle.  Small payloads (t) use
    all-same-dest slots wherever allowed: no dummy descriptors, and the
    replicated wire is negligible.  The 1 KB/partition zw payload keeps
    4 copies (halves the wire).  Cross-die dests (dq=4) may only occupy
    the D2D-capable slots 4-7.  Receiver sem increments: 2 per real slot."""
    if dq == 4:
        return [None, None, None, None, (0, dq), (0, dq), (0, dq), (0, dq)]
    if big:
        return [(0, dq)] * 4 + [None] * 4
    return [(0, dq)] * 8


def _rinc(dq, big):
    return 8 if (dq == 4 or big) else 16


def _program(tc, x, W, out):
    nc = tc.nc
    # Per-hop arrival sems (a shared counter would be ambiguous: a fast
    # partner's hop-2 arrival must not satisfy a hop-1 wait).
    rsem_t = [nc.alloc_semaphore(f"rsem_t{k}") for k in range(3)]
    rsem_z = [nc.alloc_semaphore(f"rsem_z{k}") for k in range(3)]
    # Local (send-drained) sems, one per SWDGE queue.
    lsem = [nc.alloc_semaphore(f"lsem_q{q}") for q in range(4)]
    with (
        tc.tile_pool(name="persist", bufs=1) as pp,
        tc.tile_pool(name="work", bufs=3) as wp,
        tc.tile_pool(name="psum", bufs=1, space="PSUM") as psp,
        tc.tile_pool(name="psumw", bufs=4, space="PSUM") as psw,
        tc.tile_pool(name="dram", bufs=1, space="DRAM") as dp,
    ):
        # Dangling 1-byte AllGather: marks the NEFF as collective so the
        # runtime gang-launches the 8 cores; nothing waits on it.
        cc_in = dp.tile([1, 1], F32)
        cc_out = dp.tile([R, 1], F32)
        nc.gpsimd.collective_compute(
            "AllGather", ALU.bypass, replica_groups=[list(range(R))],
            ins=[cc_in.opt()], outs=[cc_out.opt()],
        )

        ident = pp.tile([P, P], F32)
        masks.make_identity(nc, ident[:])
        x_all = pp.tile([P, T * D], F32)      # row tile i at [:, i*D:(i+1)*D]
        xb_all = pp.tile([P, T * D], BF16)    # bf16 copy of x
        g_all = pp.tile([P, T * D], BF16)     # f * x (bf16)
        xT_all = pp.tile([P, 2 * NL], BF16)   # x.T chunk c at [:, c*NL + i*P]
        W_sb = pp.tile([P, 2 * D], F32)       # W k-chunk kc at [:, kc*D]
        Wb_sb = pp.tile([P, 2 * D], BF16)

        ss = pp.tile([P, T], F32)
        invn = pp.tile([P, T], F32)
        nrm = pp.tile([P, T], F32)
        stl = pp.tile([P, T], F32)
        s_t = pp.tile([P, T], F32)
        sq_s = pp.tile([P, T], F32)
        dd = pp.tile([P, T], F32)
        f_t = pp.tile([P, T], F32)


        # Exchange buffers.  t_col / zw_loc accumulate in place; thr/zhr
        # receive the partner's running sum each hop.
        t_col = pp.tile([P, TSLOT], F32)      # my colsum partial, cols 0-1
        thr = [pp.tile([P, TSLOT], F32, name=f"thr{k}") for k in range(3)]
        ta = [t_col] + [pp.tile([P, TSLOT], F32, name=f"ta{k}") for k in range(3)]
        zw_loc = pp.tile([P, ZSLOT], BF16)    # my (z @ W) partial
        zhr = [pp.tile([P, ZSLOT], BF16, name=f"zhr{k}") for k in range(3)]
        za = [zw_loc] + [pp.tile([P, ZSLOT], BF16, name=f"za{k}") for k in range(3)]

        nc.gpsimd.memset(t_col[:], 0.0)

        # Hop k of both reductions lives on SWDGE queue k: a queue never
        # hosts two consecutive hops, so one hop's background dummy-lane
        # trickle cannot delay the next hop's descriptors.  The t-hop
        # preps are emitted here (desc-gen only, reads deferred).
        for k in range(3):
            nc.gpsimd.remote_dma_broadcast(
                thr[k][:], ta[k][:], rsem_t[k], lsem[k],
                rdests=_rdests(HOPS[k], False), queue_num=k,
            )

        for kc in range(2):
            nc.sync.dma_start(W_sb[:, kc * D:(kc + 1) * D], W[kc * P:(kc + 1) * P, :])
        nc.vector.tensor_copy(Wb_sb[:], W_sb[:])

        # ---- phase A: load shard, row norms, colsum(xn) partial ----
        from concourse.bass_types import AP as _AP
        for i in range(T):
            xs = x_all[:, i * D:(i + 1) * D]
            nc.sync.dma_start(xs, x[i * P:(i + 1) * P, :])
            scr = wp.tile([P, D], F32, tag="scr", name=f"scr{i}")
            nc.scalar.activation(scr[:], xs, AF.Square, accum_out=ss[:, i:i + 1])
            nc.vector.tensor_copy(xb_all[:, i * D:(i + 1) * D], xs)
        nc.scalar.activation(nrm[:], ss[:], AF.Sqrt)
        nc.vector.reciprocal(invn[:], nrm[:])
        invn_b = pp.tile([P, T], BF16)
        nc.vector.tensor_copy(invn_b[:], invn[:])

        # t partial in column layout: t_col[p, c] = sum_i (x_i chunk c)^T invn_i
        # (bf16 operands: one-pass matmuls, ~3x faster than fp32)
        psum_tc = psw.tile([P, D], F32, tag="pw", name="psum_tc")
        for c in range(2):
            for i in range(T):
                nc.tensor.matmul(
                    psum_tc[:, c:c + 1],
                    lhsT=xb_all[:, i * D + c * P:i * D + (c + 1) * P],
                    rhs=invn_b[:, i:i + 1],
                    start=(i == 0), stop=(i == T - 1),
                )
        nc.vector.tensor_copy(t_col[:, 0:2], psum_tc[:, 0:2])

        # x.T via PE transposes (independent work that overlaps the exchange)
        for i in range(T):
            for c in range(2):
                pt = psw.tile([P, P], F32, tag="pw", name=f"pt{i}_{c}")
                nc.tensor.transpose(
                    pt[:], x_all[:, i * D + c * P: i * D + (c + 1) * P], ident[:]
                )
                nc.vector.tensor_copy(xT_all[:, c * NL + i * P: c * NL + (i + 1) * P], pt[:])

        # y = x @ W per tile, also during the exchange window.  The zw
        # partial is then g^T @ y directly — no zT staging roundtrip.
        y_all = pp.tile([P, T * D], BF16)
        for i in range(T):
            py = psw.tile([P, D], F32, tag="pw", name=f"py{i}")
            for c in range(2):
                nc.tensor.matmul(
                    py[:], lhsT=xT_all[:, c * NL + i * P:c * NL + (i + 1) * P],
                    rhs=Wb_sb[:, c * D:(c + 1) * D],
                    start=(c == 0), stop=(c == 1),
                )
            nc.scalar.mul(y_all[:, i * D:(i + 1) * D], py[:], 1.0)

        # ---- t recursive doubling.  No entry barrier is needed:
        # target_bir_lowering is off so there is no per-kernel sem clear —
        # sems are zeroed at NEFF load and remote increments persist even if
        # a peer has not started executing yet.  Each trigger's
        # signals_writable gives it a WAW edge after the t_col producer (the
        # preps predate the producer, so the deferred-RAW edge never forms),
        # orders the hop's consumer add after it, and pins the next hop's
        # prep behind it in the queue-0 FIFO.
        add_t = []
        for k in range(3):
            # signals: WAW after the ta[k] producer, before the thr[k]
            # consumer, and pinning queue-k's zw prep (writes zhr[k])
            # behind this trigger in the FIFO.
            nc.gpsimd.trigger_dma(
                count=None, queue_num=k,
                signals_writable=(ta[k][:], thr[k][:], zhr[k][:]),
            )
            # Double-buffered: the sum lands in a fresh tile, so the hop's
            # in-flight send never races the accumulation (no local-sem wait).
            add_t.append(nc.vector.tensor_add(ta[k + 1][:], ta[k][:], thr[k][:]))

        # zw-hop send descriptors, queue k (pinned behind t-trigger k).
        for k, dq in enumerate(HOPS):
            nc.gpsimd.remote_dma_broadcast(
                zhr[k][:], za[k][:], rsem_z[k], lsem[k],
                rdests=_rdests(dq, True), queue_num=k,
            )

        # ---- phase B: degrees, f, g = f*x, zT partial, zw partial ----
        # Per-row dot products s = x . t directly on the PE using the
        # already-transposed x: psum_s[:, i] = sum_c xT(c,i)^T @ t_col_b[c]
        # — no 128-partition broadcast of t and no big multiply+reduce.
        tcol_b = pp.tile([P, 2], BF16)
        nc.vector.tensor_copy(tcol_b[:], ta[3][:, 0:2])
        psum_s = psw.tile([P, D], F32, tag="pw", name="psum_s")
        for i in range(T):
            for c in range(2):
                nc.tensor.matmul(
                    psum_s[:, i:i + 1],
                    lhsT=xT_all[:, c * NL + i * P:c * NL + (i + 1) * P],
                    rhs=tcol_b[:, c:c + 1],
                    start=(c == 0), stop=(c == 1),
                )
        nc.vector.tensor_copy(stl[:], psum_s[:, 0:T])
        nc.vector.tensor_mul(s_t[:], stl[:], invn[:])       # s = rowsum * invn
        nc.scalar.activation(sq_s[:], s_t[:], AF.Sqrt)
        nc.vector.reciprocal(dd[:], sq_s[:])                # d = rsqrt(s)
        nc.vector.tensor_mul(f_t[:], dd[:], invn[:])        # f = d * invn
        # Split the per-row scaling g = f*x across ACT and DVE so neither
        # engine serializes all eight tiles.
        for i in range(T):
            gs = g_all[:, i * D:(i + 1) * D]
            xs = x_all[:, i * D:(i + 1) * D]
            if i % 2 == 0:
                nc.scalar.mul(gs, xs, f_t[:, i:i + 1])
            else:
                nc.vector.tensor_scalar_mul(gs, xs, f_t[:, i:i + 1])

        # zw partial = (g^T y) chunk m, accumulated over the row tiles.
        psum_zw0 = psp.tile([P, D], F32, name="pzw0")
        psum_zw1 = psp.tile([P, D], F32, name="pzw1")
        for i in range(T):
            for m, pz in ((0, psum_zw0), (1, psum_zw1)):
                nc.tensor.matmul(
                    pz[:], lhsT=g_all[:, i * D + m * P: i * D + (m + 1) * P],
                    rhs=y_all[:, i * D:(i + 1) * D],
                    start=(i == 0), stop=(i == T - 1),
                )
        for m, pz in ((0, psum_zw0), (1, psum_zw1)):
            nc.vector.tensor_copy(zw_loc[:, m * D:(m + 1) * D], pz[:])

        # ---- zw recursive doubling (hops on queues 1-3) ----
        add_z = []
        for k in range(3):
            nc.gpsimd.trigger_dma(count=None, queue_num=k,
                                  signals_writable=(za[k][:], zhr[k][:]))
            add_z.append(nc.vector.tensor_add(za[k + 1][:], za[k][:], zhr[k][:]))

        # ---- phase C: out = f * (x @ zw) ----
        for i in range(T):
            po = psw.tile([P, D], F32, tag="pw", name=f"po{i}")
            for ka in range(2):
                nc.tensor.matmul(
                    po[:], lhsT=xT_all[:, ka * NL + i * P: ka * NL + (i + 1) * P],
                    rhs=za[3][:, ka * D:(ka + 1) * D],
                    start=(ka == 0), stop=(ka == 1),
                )
            o_sb = wp.tile([P, D], F32, tag="osb", name=f"osb{i}")
            nc.scalar.mul(o_sb[:], po[:], f_t[:, i:i + 1])
            nc.sync.dma_start(out[i * P:(i + 1) * P, :], o_sb[:])

    return {"add_t": add_t, "add_z": add_z,
            "rsem_t": rsem_t, "rsem_z": rsem_z, "lsem": lsem}


def _build():
    nc = bacc.Bacc("TRN2", target_bir_lowering=False, debug=False, num_devices=R,
                   num_swdge_queues=4)
    x = nc.dram_tensor("x", [NL, D], F32, kind="ExternalInput")
    W = nc.dram_tensor("W", [D, D], F32, kind="ExternalInput")
    out = nc.dram_tensor("out", [NL, D], F32, kind="ExternalOutput")
    with tile.TileContext(nc) as tc:
        h = _program(tc, x.ap() if hasattr(x, "ap") else x, W.ap() if hasattr(W, "ap") else W, out.ap() if hasattr(out, "ap") else out)
    # Attach the cross-core waits after scheduling (the schedule-time
    # single-core sim cannot model peer sem increments, and added waits
    # only delay — they cannot invalidate the schedule).  Each hop's add
    # waits for the partner's payload (+2 on the hop's remote sem) and for
    # this core's own send of the hop to drain (+16 on the queue's local
    # sem) before overwriting the send buffer.  compile() splits
    # multi-wait instructions into event semaphores automatically.
    for k in range(3):
        h["add_t"][k].wait_op(h["rsem_t"][k], _rinc(HOPS[k], False), "sem-ge", check=False)
        h["add_z"][k].wait_op(h["rsem_z"][k], _rinc(HOPS[k], True), "sem-ge", check=False)
    nc.finalize()
    return nc


def _run(inputs, trace=False):
    if "nc" not in _cache:
        _cache["nc"] = _build()
    nc = _cache["nc"]
    x = np.ascontiguousarray(inputs["x"], dtype=np.float32)
    W = np.ascontiguousarray(inputs["W"], dtype=np.float32)
    in_maps = [{"x": x[r * NL:(r + 1) * NL], "W": W} for r in range(R)]
    res = bass_utils.run_bass_kernel_spmd(
        nc, in_maps, core_ids=list(range(R)), trace=trace,
    )
    out = np.concatenate([res.results[r]["out"] for r in range(R)], axis=0)
    return out, res


def kernel(**inputs) -> np.ndarray:
    out, _ = _run(inputs, trace=False)
    return out


# revision 71
# speedup vs baseline: 1.1597x; 1.0705x over previous
"""Distributed Trainium2 kernel for the dense-graph GNN layer.

Math: with xn = x/||x|| (rows), G = xn@xn.T, d = rsqrt(G@1),
out = (diag(d) G diag(d) x) W.  The N x N Gram matrix is never needed:
  G @ 1        = xn @ t,            t = colsum(xn)            [D]
  diag(d) G diag(d) x = f * (x @ z),  z = x.T @ diag(f) @ x   [D, D]
  f_i = d_i / ||x_i||   (combines both scalings; z is symmetric)
  out = f * (x @ (z @ W))
Each core processes its 1024-row shard; the only cross-core traffic is
(1) a reduction of the [D] colsum partial and (2) a reduction of the
[D, D] (z @ W) partial.  Both reductions run as recursive-doubling
exchanges over direct peer-to-peer SBUF remote DMAs (XOR partners 1, 2,
4), which avoids both the ~70us ncfw collective bringup and the
descriptor flood of a full mesh.  A dangling 1-byte AllGather marks the
NEFF as collective so the runtime gang-launches the 8 cores (without
it, dispatch is staggered by milliseconds).
"""

import os
import sys

import numpy as np

for _p in ("/opt/trn_rl_repo", "/root/.axon_site/_ro/trn_rl_repo"):
    if os.path.isdir(_p) and _p not in sys.path:
        sys.path.insert(0, _p)

import concourse.bacc as bacc
import concourse.mybir as mybir
import concourse.tile as tile
import concourse.masks as masks
from concourse import bass_utils

R = 8                 # cores
N, D = 8192, 256
NL = N // R           # 1024 rows per core
P = 128
T = NL // P           # 8 row tiles per core
F32 = mybir.dt.float32
BF16 = mybir.dt.bfloat16
AF = mybir.ActivationFunctionType
ALU = mybir.AluOpType

TSLOT = 8             # t exchange payload width (f32 cols) = 32 B/partition
ZSLOT = 2 * D         # zw exchange payload width (bf16 cols) = 1 KB/partition
HOPS = (4, 2, 1)      # recursive-doubling XOR distances (cross-die first)

_cache = {}


def _rdests(dq, big):
    """8-slot dest list.  Every real slot ships a full 128-partition copy
    on its two lanes (64 descriptors/lane — invariant), so slots trade
    replication wire against dummy-lane trickle.  Small payloads (t) use
    all-same-dest slots wherever allowed: no dummy descriptors, and the
    replicated wire is negligible.  The 1 KB/partition zw payload keeps
    4 copies (halves the wire).  Cross-die dests (dq=4) may only occupy
    the D2D-capable slots 4-7.  Receiver sem increments: 2 per real slot."""
    if dq == 4:
        return [None, None, None, None, (0, dq), (0, dq), (0, dq), (0, dq)]
    if big:
        return [(0, dq)] * 4 + [None] * 4
    return [(0, dq)] * 8


def _rinc(dq, big):
    return 8 if (dq == 4 or big) else 16


def _program(tc, x, W, out):
    nc = tc.nc
    # Per-hop arrival sems (a shared counter would be ambiguous: a fast
    # partner's hop-2 arrival must not satisfy a hop-1 wait).
    rsem_t = [nc.alloc_semaphore(f"rsem_t{k}") for k in range(3)]
    rsem_z = [nc.alloc_semaphore(f"rsem_z{k}") for k in range(3)]
    # Local (send-drained) sems, one per SWDGE queue.
    lsem = [nc.alloc_semaphore(f"lsem_q{q}") for q in range(4)]
    with (
        tc.tile_pool(name="persist", bufs=1) as pp,
        tc.tile_pool(name="work", bufs=3) as wp,
        tc.tile_pool(name="psum", bufs=1, space="PSUM") as psp,
        tc.tile_pool(name="psumw", bufs=4, space="PSUM") as psw,
        tc.tile_pool(name="dram", bufs=1, space="DRAM") as dp,
    ):
        # Dangling 1-byte AllGather: marks the NEFF as collective so the
        # runtime gang-launches the 8 cores; nothing waits on it.
        cc_in = dp.tile([1, 1], F32)
        cc_out = dp.tile([R, 1], F32)
        nc.gpsimd.collective_compute(
            "AllGather", ALU.bypass, replica_groups=[list(range(R))],
            ins=[cc_in.opt()], outs=[cc_out.opt()],
        )

        ident = pp.tile([P, P], F32)
        masks.make_identity(nc, ident[:])
        x_all = pp.tile([P, T * D], F32)      # row tile i at [:, i*D:(i+1)*D]
        xb_all = pp.tile([P, T * D], BF16)    # bf16 copy of x
        g_all = pp.tile([P, T * D], BF16)     # f * x (bf16)
        xT_all = pp.tile([P, 2 * NL], BF16)   # x.T chunk c at [:, c*NL + i*P]
        W_sb = pp.tile([P, 2 * D], F32)       # W k-chunk kc at [:, kc*D]
        Wb_sb = pp.tile([P, 2 * D], BF16)

        ss = pp.tile([P, T], F32)
        invn = pp.tile([P, T], F32)
        nrm = pp.tile([P, T], F32)
        stl = pp.tile([P, T], F32)
        s_t = pp.tile([P, T], F32)
        sq_s = pp.tile([P, T], F32)
        dd = pp.tile([P, T], F32)
        f_t = pp.tile([P, T], F32)


        # Exchange buffers.  t_col / zw_loc accumulate in place; thr/zhr
        # receive the partner's running sum each hop.
        t_col = pp.tile([P, TSLOT], F32)      # my colsum partial, cols 0-1
        thr = [pp.tile([P, TSLOT], F32, name=f"thr{k}") for k in range(3)]
        ta = [t_col] + [pp.tile([P, TSLOT], F32, name=f"ta{k}") for k in range(3)]
        zw_loc = pp.tile([P, ZSLOT], BF16)    # my (z @ W) partial
        zhr = [pp.tile([P, ZSLOT], BF16, name=f"zhr{k}") for k in range(3)]
        za = [zw_loc] + [pp.tile([P, ZSLOT], BF16, name=f"za{k}") for k in range(3)]

        nc.gpsimd.memset(t_col[:], 0.0)

        # Hop k of both reductions lives on SWDGE queue k: a queue never
        # hosts two consecutive hops, so one hop's background dummy-lane
        # trickle cannot delay the next hop's descriptors.  The t-hop
        # preps are emitted here (desc-gen only, reads deferred).
        for k in range(3):
            nc.gpsimd.remote_dma_broadcast(
                thr[k][:], ta[k][:], rsem_t[k], lsem[k],
                rdests=_rdests(HOPS[k], False), queue_num=k,
            )

        for kc in range(2):
            nc.sync.dma_start(W_sb[:, kc * D:(kc + 1) * D], W[kc * P:(kc + 1) * P, :])
        nc.vector.tensor_copy(Wb_sb[:], W_sb[:])

        # ---- phase A: load shard, row norms, colsum(xn) partial ----
        from concourse.bass_types import AP as _AP
        for i in range(T):
            xs = x_all[:, i * D:(i + 1) * D]
            nc.sync.dma_start(xs, x[i * P:(i + 1) * P, :])
            scr = wp.tile([P, D], F32, tag="scr", name=f"scr{i}")
            nc.scalar.activation(scr[:], xs, AF.Square, accum_out=ss[:, i:i + 1])
            nc.vector.tensor_copy(xb_all[:, i * D:(i + 1) * D], xs)
        nc.scalar.activation(nrm[:], ss[:], AF.Sqrt)
        nc.vector.reciprocal(invn[:], nrm[:])
        invn_b = pp.tile([P, T], BF16)
        nc.vector.tensor_copy(invn_b[:], invn[:])

        # t partial in column layout: t_col[p, c] = sum_i (x_i chunk c)^T invn_i
        # (bf16 operands: one-pass matmuls, ~3x faster than fp32)
        psum_tc = psw.tile([P, D], F32, tag="pw", name="psum_tc")
        for c in range(2):
            for i in range(T):
                nc.tensor.matmul(
                    psum_tc[:, c:c + 1],
                    lhsT=xb_all[:, i * D + c * P:i * D + (c + 1) * P],
                    rhs=invn_b[:, i:i + 1],
                    start=(i == 0), stop=(i == T - 1),
                )
        nc.vector.tensor_copy(t_col[:, 0:2], psum_tc[:, 0:2])

        # x.T via PE transposes (independent work that overlaps the exchange)
        for i in range(T):
            for c in range(2):
                pt = psw.tile([P, P], F32, tag="pw", name=f"pt{i}_{c}")
                nc.tensor.transpose(
                    pt[:], x_all[:, i * D + c * P: i * D + (c + 1) * P], ident[:]
                )
                nc.vector.tensor_copy(xT_all[:, c * NL + i * P: c * NL + (i + 1) * P], pt[:])

        # y = x @ W per tile, also during the exchange window.  The zw
        # partial is then g^T @ y directly — no zT staging roundtrip.
        y_all = pp.tile([P, T * D], BF16)
        for i in range(T):
            py = psw.tile([P, D], F32, tag="pw", name=f"py{i}")
            for c in range(2):
                nc.tensor.matmul(
                    py[:], lhsT=xT_all[:, c * NL + i * P:c * NL + (i + 1) * P],
                    rhs=Wb_sb[:, c * D:(c + 1) * D],
                    start=(c == 0), stop=(c == 1),
                )
            nc.scalar.mul(y_all[:, i * D:(i + 1) * D], py[:], 1.0)

        # ---- t recursive doubling.  No entry barrier is needed:
        # target_bir_lowering is off so there is no per-kernel sem clear —
        # sems are zeroed at NEFF load and remote increments persist even if
        # a peer has not started executing yet.  Each trigger's
        # signals_writable gives it a WAW edge after the t_col producer (the
        # preps predate the producer, so the deferred-RAW edge never forms),
        # orders the hop's consumer add after it, and pins the next hop's
        # prep behind it in the queue-0 FIFO.
        add_t = []
        for k in range(3):
            # signals: WAW after the ta[k] producer, before the thr[k]
            # consumer, and pinning queue-k's zw prep (writes zhr[k])
            # behind this trigger in the FIFO.
            nc.gpsimd.trigger_dma(
                count=None, queue_num=k,
                signals_writable=(ta[k][:], thr[k][:], zhr[k][:]),
            )
            # Double-buffered: the sum lands in a fresh tile, so the hop's
            # in-flight send never races the accumulation (no local-sem wait).
            add_t.append(nc.vector.tensor_add(ta[k + 1][:], ta[k][:], thr[k][:]))

        # zw-hop send descriptors, queue k (pinned behind t-trigger k).
        for k, dq in enumerate(HOPS):
            nc.gpsimd.remote_dma_broadcast(
                zhr[k][:], za[k][:], rsem_z[k], lsem[k],
                rdests=_rdests(dq, True), queue_num=k,
            )

        # ---- phase B: degrees, f, g = f*x, zT partial, zw partial ----
        # Per-row dot products s = x . t directly on the PE using the
        # already-transposed x: psum_s[:, i] = sum_c xT(c,i)^T @ t_col_b[c]
        # — no 128-partition broadcast of t and no big multiply+reduce.
        tcol_b = pp.tile([P, 2], BF16)
        nc.vector.tensor_copy(tcol_b[:], ta[3][:, 0:2])
        psum_s = psw.tile([P, D], F32, tag="pw", name="psum_s")
        for i in range(T):
            for c in range(2):
                nc.tensor.matmul(
                    psum_s[:, i:i + 1],
                    lhsT=xT_all[:, c * NL + i * P:c * NL + (i + 1) * P],
                    rhs=tcol_b[:, c:c + 1],
                    start=(c == 0), stop=(c == 1),
                )
        nc.vector.tensor_copy(stl[:], psum_s[:, 0:T])
        nc.vector.tensor_mul(s_t[:], stl[:], invn[:])       # s = rowsum * invn
        nc.scalar.activation(sq_s[:], s_t[:], AF.Sqrt)
        nc.vector.reciprocal(dd[:], sq_s[:])                # d = rsqrt(s)
        nc.vector.tensor_mul(f_t[:], dd[:], invn[:])        # f = d * invn
        # Split the per-row scaling g = f*x across ACT and DVE so neither
        # engine serializes all eight tiles.
        for i in range(T):
            gs = g_all[:, i * D:(i + 1) * D]
            xs = x_all[:, i * D:(i + 1) * D]
            if i % 2 == 0:
                nc.scalar.mul(gs, xs, f_t[:, i:i + 1])
            else:
                nc.vector.tensor_scalar_mul(gs, xs, f_t[:, i:i + 1])

        # zw partial = (g^T y) chunk m, accumulated over the row tiles.
        psum_zw0 = psp.tile([P, D], F32, name="pzw0")
        psum_zw1 = psp.tile([P, D], F32, name="pzw1")
        for i in range(T):
            for m, pz in ((0, psum_zw0), (1, psum_zw1)):
                nc.tensor.matmul(
                    pz[:], lhsT=g_all[:, i * D + m * P: i * D + (m + 1) * P],
                    rhs=y_all[:, i * D:(i + 1) * D],
                    start=(i == 0), stop=(i == T - 1),
                )
        for m, pz in ((0, psum_zw0), (1, psum_zw1)):
            nc.vector.tensor_copy(zw_loc[:, m * D:(m + 1) * D], pz[:])

        # ---- zw recursive doubling (hops on queues 1-3) ----
        add_z = []
        for k in range(3):
            nc.gpsimd.trigger_dma(count=None, queue_num=k,
                                  signals_writable=(za[k][:], zhr[k][:]))
            add_z.append(nc.vector.tensor_add(za[k + 1][:], za[k][:], zhr[k][:]))

        # ---- phase C: out = f * (x @ zw) ----
        for i in range(T):
            po = psw.tile([P, D], F32, tag="pw", name=f"po{i}")
            for ka in range(2):
                nc.tensor.matmul(
                    po[:], lhsT=xT_all[:, ka * NL + i * P: ka * NL + (i + 1) * P],
                    rhs=za[3][:, ka * D:(ka + 1) * D],
                    start=(ka == 0), stop=(ka == 1),
                )
            o_sb = wp.tile([P, D], F32, tag="osb", name=f"osb{i}")
            nc.scalar.mul(o_sb[:], po[:], f_t[:, i:i + 1])
            nc.sync.dma_start(out[i * P:(i + 1) * P, :], o_sb[:])

    return {"add_t": add_t, "add_z": add_z,
            "rsem_t": rsem_t, "rsem_z": rsem_z, "lsem": lsem}


def _build():
    nc = bacc.Bacc("TRN2", target_bir_lowering=False, debug=False, num_devices=R,
                   num_swdge_queues=4)
    x = nc.dram_tensor("x", [NL, D], F32, kind="ExternalInput")
    W = nc.dram_tensor("W", [D, D], F32, kind="ExternalInput")
    out = nc.dram_tensor("out", [NL, D], F32, kind="ExternalOutput")
    with tile.TileContext(nc) as tc:
        h = _program(tc, x.ap() if hasattr(x, "ap") else x, W.ap() if hasattr(W, "ap") else W, out.ap() if hasattr(out, "ap") else out)
    # Attach the cross-core waits after scheduling (the schedule-time
    # single-core sim cannot model peer sem increments, and added waits
    # only delay — they cannot invalidate the schedule).  Each hop's add
    # waits for the partner's payload (+2 on the hop's remote sem) and for
    # this core's own send of the hop to drain (+16 on the queue's local
    # sem) before overwriting the send buffer.  compile() splits
    # multi-wait instructions into event semaphores automatically.
    for k in range(3):
        h["add_t"][k].wait_op(h["rsem_t"][k], _rinc(HOPS[k], False), "sem-ge", check=False)
        h["add_z"][k].wait_op(h["rsem_z"][k], _rinc(HOPS[k], True), "sem-ge", check=False)
    nc.finalize()
    return nc


def _run(inputs, trace=False):
    if "nc" not in _cache:
        _cache["nc"] = _build()
    nc = _cache["nc"]
    x = np.ascontiguousarray(inputs["x"], dtype=np.float32)
    W = np.ascontiguousarray(inputs["W"], dtype=np.float32)
    in_maps = [{"x": x[r * NL:(r + 1) * NL], "W": W} for r in range(R)]
    res = bass_utils.run_bass_kernel_spmd(
        nc, in_maps, core_ids=list(range(R)), trace=trace,
    )
    out = np.concatenate([res.results[r]["out"] for r in range(R)], axis=0)
    return out, res


def kernel(**inputs) -> np.ndarray:
    out, _ = _run(inputs, trace=False)
    return out


# revision 72
# speedup vs baseline: 1.1636x; 1.0033x over previous
"""Distributed Trainium2 kernel for the dense-graph GNN layer.

Math: with xn = x/||x|| (rows), G = xn@xn.T, d = rsqrt(G@1),
out = (diag(d) G diag(d) x) W.  The N x N Gram matrix is never needed:
  G @ 1        = xn @ t,            t = colsum(xn)            [D]
  diag(d) G diag(d) x = f * (x @ z),  z = x.T @ diag(f) @ x   [D, D]
  f_i = d_i / ||x_i||   (combines both scalings; z is symmetric)
  out = f * (x @ (z @ W)) = f * (x @ (x.T @ (f*x) @ W))
Each core processes its 1024-row shard; the only cross-core traffic is
(1) a reduction of the [D] colsum partial and (2) a reduction of the
[D, D] (z @ W) partial.  Both reductions run as recursive-doubling
exchanges over direct peer-to-peer SBUF remote DMAs (XOR partners
4, 2, 1 — cross-die first, so the launch-skew wait overlaps it), which
avoids the ~70us ncfw collective bringup entirely.  Hop accumulation is
double-buffered so an in-flight send never races the next sum.  A
dangling 1-byte AllGather marks the NEFF as collective so the runtime
gang-launches the 8 cores (without it, dispatch is staggered by
milliseconds).  The cross-core arrival waits are attached to the hop
adds after Tile scheduling — the schedule-time single-core sim cannot
model peer sem increments, and target_bir_lowering=False means sems are
zeroed at NEFF load (not per execution), so remote increments persist
even if a peer has not started executing yet and no entry barrier is
needed.
"""

import os
import sys

import numpy as np

for _p in ("/opt/trn_rl_repo", "/root/.axon_site/_ro/trn_rl_repo"):
    if os.path.isdir(_p) and _p not in sys.path:
        sys.path.insert(0, _p)

import concourse.bacc as bacc
import concourse.mybir as mybir
import concourse.tile as tile
import concourse.masks as masks
from concourse import bass_utils

R = 8                 # cores
N, D = 8192, 256
NL = N // R           # 1024 rows per core
P = 128
T = NL // P           # 8 row tiles per core
F32 = mybir.dt.float32
BF16 = mybir.dt.bfloat16
AF = mybir.ActivationFunctionType
ALU = mybir.AluOpType

TSLOT = 8             # t exchange payload width (f32 cols) = 32 B/partition
ZSLOT = 2 * D         # zw exchange payload width (bf16 cols) = 1 KB/partition
HOPS = (4, 2, 1)      # recursive-doubling XOR distances (cross-die first)

_cache = {}


def _rdests(dq, big):
    """8-slot dest list.  Every real slot ships a full 128-partition copy
    on its two lanes (64 descriptors/lane — invariant), so slots trade
    replication wire against dummy-lane trickle.  Small payloads (t) use
    all-same-dest slots wherever allowed: no dummy descriptors, and the
    replicated wire is negligible.  The 1 KB/partition zw payload keeps
    4 copies (halves the wire).  Cross-die dests (dq=4) may only occupy
    the D2D-capable slots 4-7.  Receiver sem increments: 2 per real slot."""
    if dq == 4:
        return [None, None, None, None, (0, dq), (0, dq), (0, dq), (0, dq)]
    if big:
        return [(0, dq)] * 4 + [None] * 4
    return [(0, dq)] * 8


def _rinc(dq, big):
    return 8 if (dq == 4 or big) else 16


def _program(tc, x, W, out):
    nc = tc.nc
    # Per-hop arrival sems (a shared counter would be ambiguous: a fast
    # partner's hop-2 arrival must not satisfy a hop-1 wait).
    rsem_t = [nc.alloc_semaphore(f"rsem_t{k}") for k in range(3)]
    rsem_z = [nc.alloc_semaphore(f"rsem_z{k}") for k in range(3)]
    # Local (send-drained) sems, one per SWDGE queue.
    lsem = [nc.alloc_semaphore(f"lsem_q{q}") for q in range(4)]
    with (
        tc.tile_pool(name="persist", bufs=1) as pp,
        tc.tile_pool(name="work", bufs=3) as wp,
        tc.tile_pool(name="psum", bufs=1, space="PSUM") as psp,
        tc.tile_pool(name="psumw", bufs=4, space="PSUM") as psw,
        tc.tile_pool(name="dram", bufs=1, space="DRAM") as dp,
    ):
        # Dangling 1-byte AllGather: marks the NEFF as collective so the
        # runtime gang-launches the 8 cores; nothing waits on it.
        cc_in = dp.tile([1, 1], F32)
        cc_out = dp.tile([R, 1], F32)
        nc.gpsimd.collective_compute(
            "AllGather", ALU.bypass, replica_groups=[list(range(R))],
            ins=[cc_in.opt()], outs=[cc_out.opt()],
        )

        ident = pp.tile([P, P], F32)
        masks.make_identity(nc, ident[:])
        x_all = pp.tile([P, T * D], F32)      # row tile i at [:, i*D:(i+1)*D]
        xb_all = pp.tile([P, T * D], BF16)    # bf16 copy of x
        g_all = pp.tile([P, T * D], BF16)     # f * x (bf16)
        xT_all = pp.tile([P, 2 * NL], BF16)   # x.T chunk c at [:, c*NL + i*P]
        W_sb = pp.tile([P, 2 * D], F32)       # W k-chunk kc at [:, kc*D]
        Wb_sb = pp.tile([P, 2 * D], BF16)

        ss = pp.tile([P, T], F32)
        invn = pp.tile([P, T], F32)
        nrm = pp.tile([P, T], F32)
        stl = pp.tile([P, T], F32)
        s_t = pp.tile([P, T], F32)
        sq_s = pp.tile([P, T], F32)
        dd = pp.tile([P, T], F32)
        f_t = pp.tile([P, T], F32)


        # Exchange buffers.  t_col / zw_loc accumulate in place; thr/zhr
        # receive the partner's running sum each hop.
        t_col = pp.tile([P, TSLOT], F32)      # my colsum partial, cols 0-1
        thr = [pp.tile([P, TSLOT], F32, name=f"thr{k}") for k in range(3)]
        ta = [t_col] + [pp.tile([P, TSLOT], F32, name=f"ta{k}") for k in range(3)]
        zw_loc = pp.tile([P, ZSLOT], BF16)    # my (z @ W) partial
        zhr = [pp.tile([P, ZSLOT], BF16, name=f"zhr{k}") for k in range(3)]
        za = [zw_loc] + [pp.tile([P, ZSLOT], BF16, name=f"za{k}") for k in range(3)]

        nc.gpsimd.memset(t_col[:], 0.0)

        # Hop k of both reductions lives on SWDGE queue k: a queue never
        # hosts two consecutive hops, so one hop's background dummy-lane
        # trickle cannot delay the next hop's descriptors.  The t-hop
        # preps are emitted here (desc-gen only, reads deferred).
        for k in range(3):
            nc.gpsimd.remote_dma_broadcast(
                thr[k][:], ta[k][:], rsem_t[k], lsem[k],
                rdests=_rdests(HOPS[k], False), queue_num=k,
            )

        for kc in range(2):
            nc.sync.dma_start(W_sb[:, kc * D:(kc + 1) * D], W[kc * P:(kc + 1) * P, :])
        nc.vector.tensor_copy(Wb_sb[:], W_sb[:])

        # ---- phase A: load shard, row norms, colsum(xn) partial ----
        from concourse.bass_types import AP as _AP
        for i in range(T):
            xs = x_all[:, i * D:(i + 1) * D]
            nc.sync.dma_start(xs, x[i * P:(i + 1) * P, :])
            scr = wp.tile([P, D], F32, tag="scr", name=f"scr{i}")
            nc.scalar.activation(scr[:], xs, AF.Square, accum_out=ss[:, i:i + 1])
            nc.vector.tensor_copy(xb_all[:, i * D:(i + 1) * D], xs)
        nc.scalar.activation(nrm[:], ss[:], AF.Sqrt)
        nc.vector.reciprocal(invn[:], nrm[:])
        invn_b = pp.tile([P, T], BF16)
        nc.vector.tensor_copy(invn_b[:], invn[:])

        # t partial in column layout: t_col[p, c] = sum_i (x_i chunk c)^T invn_i
        # (bf16 operands: one-pass matmuls, ~3x faster than fp32)
        psum_tc = psw.tile([P, D], F32, tag="pw", name="psum_tc")
        for c in range(2):
            for i in range(T):
                nc.tensor.matmul(
                    psum_tc[:, c:c + 1],
                    lhsT=xb_all[:, i * D + c * P:i * D + (c + 1) * P],
                    rhs=invn_b[:, i:i + 1],
                    start=(i == 0), stop=(i == T - 1),
                )
        nc.vector.tensor_copy(t_col[:, 0:2], psum_tc[:, 0:2])

        # x.T via PE transposes (independent work that overlaps the exchange)
        for i in range(T):
            for c in range(2):
                pt = psw.tile([P, P], F32, tag="pw", name=f"pt{i}_{c}")
                nc.tensor.transpose(
                    pt[:], x_all[:, i * D + c * P: i * D + (c + 1) * P], ident[:]
                )
                nc.vector.tensor_copy(xT_all[:, c * NL + i * P: c * NL + (i + 1) * P], pt[:])

        # y = x @ W per tile, also during the exchange window.  The zw
        # partial is then g^T @ y directly — no zT staging roundtrip.
        y_all = pp.tile([P, T * D], BF16)
        for i in range(T):
            py = psw.tile([P, D], F32, tag="pw", name=f"py{i}")
            for c in range(2):
                nc.tensor.matmul(
                    py[:], lhsT=xT_all[:, c * NL + i * P:c * NL + (i + 1) * P],
                    rhs=Wb_sb[:, c * D:(c + 1) * D],
                    start=(c == 0), stop=(c == 1),
                )
            nc.scalar.mul(y_all[:, i * D:(i + 1) * D], py[:], 1.0)

        # ---- t recursive doubling.  No entry barrier is needed:
        # target_bir_lowering is off so there is no per-kernel sem clear —
        # sems are zeroed at NEFF load and remote increments persist even if
        # a peer has not started executing yet.  Each trigger's
        # signals_writable gives it a WAW edge after the t_col producer (the
        # preps predate the producer, so the deferred-RAW edge never forms),
        # orders the hop's consumer add after it, and pins the next hop's
        # prep behind it in the queue-0 FIFO.
        add_t = []
        for k in range(3):
            # signals: WAW after the ta[k] producer, before the thr[k]
            # consumer, and pinning queue-k's zw prep (writes zhr[k])
            # behind this trigger in the FIFO.
            nc.gpsimd.trigger_dma(
                count=None, queue_num=k,
                signals_writable=(ta[k][:], thr[k][:], zhr[k][:]),
            )
            # Double-buffered: the sum lands in a fresh tile, so the hop's
            # in-flight send never races the accumulation (no local-sem wait).
            add_t.append(nc.vector.tensor_add(ta[k + 1][:], ta[k][:], thr[k][:]))

        # zw-hop send descriptors, queue k (pinned behind t-trigger k).
        for k, dq in enumerate(HOPS):
            nc.gpsimd.remote_dma_broadcast(
                zhr[k][:], za[k][:], rsem_z[k], lsem[k],
                rdests=_rdests(dq, True), queue_num=k,
            )

        # ---- phase B: degrees, f, g = f*x, zT partial, zw partial ----
        # Per-row dot products s = x . t directly on the PE using the
        # already-transposed x: psum_s[:, i] = sum_c xT(c,i)^T @ t_col_b[c]
        # — no 128-partition broadcast of t and no big multiply+reduce.
        tcol_b = pp.tile([P, 2], BF16)
        nc.vector.tensor_copy(tcol_b[:], ta[3][:, 0:2])
        psum_s = psw.tile([P, D], F32, tag="pw", name="psum_s")
        for i in range(T):
            for c in range(2):
                nc.tensor.matmul(
                    psum_s[:, i:i + 1],
                    lhsT=xT_all[:, c * NL + i * P:c * NL + (i + 1) * P],
                    rhs=tcol_b[:, c:c + 1],
                    start=(c == 0), stop=(c == 1),
                )
        nc.vector.tensor_copy(stl[:], psum_s[:, 0:T])
        nc.vector.tensor_mul(s_t[:], stl[:], invn[:])       # s = rowsum * invn
        nc.scalar.activation(sq_s[:], s_t[:], AF.Sqrt)
        nc.vector.reciprocal(dd[:], sq_s[:])                # d = rsqrt(s)
        nc.vector.tensor_mul(f_t[:], dd[:], invn[:])        # f = d * invn
        # Split the per-row scaling g = f*x across ACT and DVE so neither
        # engine serializes all eight tiles.
        for i in range(T):
            gs = g_all[:, i * D:(i + 1) * D]
            xs = x_all[:, i * D:(i + 1) * D]
            if i % 2 == 0:
                nc.scalar.mul(gs, xs, f_t[:, i:i + 1])
            else:
                nc.vector.tensor_scalar_mul(gs, xs, f_t[:, i:i + 1])

        # zw partial = (g^T y) chunk m, accumulated over the row tiles.
        psum_zw0 = psp.tile([P, D], F32, name="pzw0")
        psum_zw1 = psp.tile([P, D], F32, name="pzw1")
        for i in range(T):
            for m, pz in ((0, psum_zw0), (1, psum_zw1)):
                nc.tensor.matmul(
                    pz[:], lhsT=g_all[:, i * D + m * P: i * D + (m + 1) * P],
                    rhs=y_all[:, i * D:(i + 1) * D],
                    start=(i == 0), stop=(i == T - 1),
                )
        for m, pz in ((0, psum_zw0), (1, psum_zw1)):
            nc.vector.tensor_copy(zw_loc[:, m * D:(m + 1) * D], pz[:])

        # ---- zw recursive doubling (hops on queues 1-3) ----
        add_z = []
        for k in range(3):
            nc.gpsimd.trigger_dma(count=None, queue_num=k,
                                  signals_writable=(za[k][:], zhr[k][:]))
            add_z.append(nc.vector.tensor_add(za[k + 1][:], za[k][:], zhr[k][:]))

        # ---- phase C: out = f * (x @ zw) ----
        for i in range(T):
            po = psw.tile([P, D], F32, tag="pw", name=f"po{i}")
            for ka in range(2):
                nc.tensor.matmul(
                    po[:], lhsT=xT_all[:, ka * NL + i * P: ka * NL + (i + 1) * P],
                    rhs=za[3][:, ka * D:(ka + 1) * D],
                    start=(ka == 0), stop=(ka == 1),
                )
            o_sb = wp.tile([P, D], F32, tag="osb", name=f"osb{i}")
            nc.scalar.mul(o_sb[:], po[:], f_t[:, i:i + 1])
            nc.sync.dma_start(out[i * P:(i + 1) * P, :], o_sb[:])

    return {"add_t": add_t, "add_z": add_z,
            "rsem_t": rsem_t, "rsem_z": rsem_z, "lsem": lsem}


def _build():
    nc = bacc.Bacc("TRN2", target_bir_lowering=False, debug=False, num_devices=R,
                   num_swdge_queues=4)
    x = nc.dram_tensor("x", [NL, D], F32, kind="ExternalInput")
    W = nc.dram_tensor("W", [D, D], F32, kind="ExternalInput")
    out = nc.dram_tensor("out", [NL, D], F32, kind="ExternalOutput")
    with tile.TileContext(nc) as tc:
        h = _program(tc, x.ap() if hasattr(x, "ap") else x, W.ap() if hasattr(W, "ap") else W, out.ap() if hasattr(out, "ap") else out)
    # Attach the cross-core waits after scheduling (the schedule-time
    # single-core sim cannot model peer sem increments, and added waits
    # only delay — they cannot invalidate the schedule).  Each hop's add
    # waits for the partner's payload (+2 on the hop's remote sem) and for
    # this core's own send of the hop to drain (+16 on the queue's local
    # sem) before overwriting the send buffer.  compile() splits
    # multi-wait instructions into event semaphores automatically.
    for k in range(3):
        h["add_t"][k].wait_op(h["rsem_t"][k], _rinc(HOPS[k], False), "sem-ge", check=False)
        h["add_z"][k].wait_op(h["rsem_z"][k], _rinc(HOPS[k], True), "sem-ge", check=False)
    nc.finalize()
    return nc


def _run(inputs, trace=False):
    if "nc" not in _cache:
        _cache["nc"] = _build()
    nc = _cache["nc"]
    x = np.ascontiguousarray(inputs["x"], dtype=np.float32)
    W = np.ascontiguousarray(inputs["W"], dtype=np.float32)
    in_maps = [{"x": x[r * NL:(r + 1) * NL], "W": W} for r in range(R)]
    res = bass_utils.run_bass_kernel_spmd(
        nc, in_maps, core_ids=list(range(R)), trace=trace,
    )
    out = np.concatenate([res.results[r]["out"] for r in range(R)], axis=0)
    return out, res


def kernel(**inputs) -> np.ndarray:
    out, _ = _run(inputs, trace=False)
    return out


# revision 74
# speedup vs baseline: 1.1921x; 1.0245x over previous
"""Distributed Trainium2 kernel for the dense-graph GNN layer.

Math: with xn = x/||x|| (rows), G = xn@xn.T, d = rsqrt(G@1),
out = (diag(d) G diag(d) x) W.  The N x N Gram matrix is never needed:
  G @ 1        = xn @ t,            t = colsum(xn)            [D]
  diag(d) G diag(d) x = f * (x @ z),  z = x.T @ diag(f) @ x   [D, D]
  f_i = d_i / ||x_i||   (combines both scalings; z is symmetric)
  out = f * (x @ (z @ W)) = f * (x @ (x.T @ (f*x) @ W))
Each core processes its 1024-row shard; the only cross-core traffic is
(1) a reduction of the [D] colsum partial and (2) a reduction of the
[D, D] (z @ W) partial.  Both reductions run as recursive-doubling
exchanges over direct peer-to-peer SBUF remote DMAs (XOR partners
4, 2, 1 — cross-die first, so the launch-skew wait overlaps it), which
avoids the ~70us ncfw collective bringup entirely.  Hop accumulation is
double-buffered so an in-flight send never races the next sum.  A
dangling 1-byte AllGather marks the NEFF as collective so the runtime
gang-launches the 8 cores (without it, dispatch is staggered by
milliseconds).  The cross-core arrival waits are attached to the hop
adds after Tile scheduling — the schedule-time single-core sim cannot
model peer sem increments, and target_bir_lowering=False means sems are
zeroed at NEFF load (not per execution), so remote increments persist
even if a peer has not started executing yet and no entry barrier is
needed.
"""

import os
import sys

import numpy as np

for _p in ("/opt/trn_rl_repo", "/root/.axon_site/_ro/trn_rl_repo"):
    if os.path.isdir(_p) and _p not in sys.path:
        sys.path.insert(0, _p)

import concourse.bacc as bacc
import concourse.mybir as mybir
import concourse.tile as tile
import concourse.masks as masks
from concourse import bass_utils

R = 8                 # cores
N, D = 8192, 256
NL = N // R           # 1024 rows per core
P = 128
T = NL // P           # 8 row tiles per core
F32 = mybir.dt.float32
BF16 = mybir.dt.bfloat16
AF = mybir.ActivationFunctionType
ALU = mybir.AluOpType

TSLOT = 8             # t exchange payload width (f32 cols) = 32 B/partition
ZSLOT = 2 * D         # zw exchange payload width (bf16 cols) = 1 KB/partition
HOPS = (4, 2, 1)      # recursive-doubling XOR distances (cross-die first)

_cache = {}


def _rdests(dq, big):
    """8-slot dest list.  Every real slot ships a full 128-partition copy
    on its two lanes (64 descriptors/lane — invariant), so slots trade
    replication wire against dummy-lane trickle.  Small payloads (t) use
    all-same-dest slots wherever allowed: no dummy descriptors, and the
    replicated wire is negligible.  The 1 KB/partition zw payload keeps
    4 copies (halves the wire).  Cross-die dests (dq=4) may only occupy
    the D2D-capable slots 4-7.  Receiver sem increments: 2 per real slot."""
    if dq == 4:
        return [None, None, None, None, (0, dq), (0, dq), (0, dq), (0, dq)]
    if big:
        return [(0, dq)] * 4 + [None] * 4
    return [(0, dq)] * 8


def _rinc(dq, big):
    return 8 if (dq == 4 or big) else 16


def _program(tc, x, W, out):
    nc = tc.nc
    # Per-hop arrival sems (a shared counter would be ambiguous: a fast
    # partner's hop-2 arrival must not satisfy a hop-1 wait).
    rsem_t = [nc.alloc_semaphore(f"rsem_t{k}") for k in range(3)]
    rsem_z = [nc.alloc_semaphore(f"rsem_z{k}") for k in range(3)]
    # Local (send-drained) sems, one per SWDGE queue.
    lsem = [nc.alloc_semaphore(f"lsem_q{q}") for q in range(4)]
    with (
        tc.tile_pool(name="persist", bufs=1) as pp,
        tc.tile_pool(name="work", bufs=3) as wp,
        tc.tile_pool(name="psum", bufs=1, space="PSUM") as psp,
        tc.tile_pool(name="psumw", bufs=4, space="PSUM") as psw,
        tc.tile_pool(name="dram", bufs=1, space="DRAM") as dp,
    ):
        # Dangling 1-byte AllGather: marks the NEFF as collective so the
        # runtime gang-launches the 8 cores; nothing waits on it.
        cc_in = dp.tile([1, 1], F32)
        cc_out = dp.tile([R, 1], F32)
        nc.gpsimd.collective_compute(
            "AllGather", ALU.bypass, replica_groups=[list(range(R))],
            ins=[cc_in.opt()], outs=[cc_out.opt()],
        )

        ident = pp.tile([P, P], F32)
        masks.make_identity(nc, ident[:])
        x_all = pp.tile([P, T * D], F32)      # row tile i at [:, i*D:(i+1)*D]
        xb_all = pp.tile([P, T * D], BF16)    # bf16 copy of x
        g_all = pp.tile([P, T * D], BF16)     # f * x (bf16)
        xT_all = pp.tile([P, 2 * NL], BF16)   # x.T chunk c at [:, c*NL + i*P]
        W_sb = pp.tile([P, 2 * D], F32)       # W k-chunk kc at [:, kc*D]
        Wb_sb = pp.tile([P, 2 * D], BF16)

        ss = pp.tile([P, T], F32)
        invn = pp.tile([P, T], F32)
        nrm = pp.tile([P, T], F32)
        stl = pp.tile([P, T], F32)
        s_t = pp.tile([P, T], F32)
        sq_s = pp.tile([P, T], F32)
        dd = pp.tile([P, T], F32)
        f_t = pp.tile([P, T], F32)


        # Exchange buffers.  t_col / zw_loc accumulate in place; thr/zhr
        # receive the partner's running sum each hop.
        t_col = pp.tile([P, TSLOT], F32)      # my colsum partial, cols 0-1
        thr = [pp.tile([P, TSLOT], F32, name=f"thr{k}") for k in range(3)]
        ta = [t_col] + [pp.tile([P, TSLOT], F32, name=f"ta{k}") for k in range(3)]
        zw_loc = pp.tile([P, ZSLOT], BF16)    # my (z @ W) partial
        F8 = mybir.dt.float8e4
        zs8 = [pp.tile([P, ZSLOT], F8, name=f"zs8_{k}") for k in range(3)]
        zhr = [pp.tile([P, ZSLOT], F8, name=f"zhr{k}") for k in range(3)]
        za = [zw_loc] + [pp.tile([P, ZSLOT], BF16, name=f"za{k}") for k in range(3)]

        nc.gpsimd.memset(t_col[:], 0.0)

        # Hop k of both reductions lives on SWDGE queue k: a queue never
        # hosts two consecutive hops, so one hop's background dummy-lane
        # trickle cannot delay the next hop's descriptors.  The t-hop
        # preps are emitted here (desc-gen only, reads deferred).
        for k in range(3):
            nc.gpsimd.remote_dma_broadcast(
                thr[k][:], ta[k][:], rsem_t[k], lsem[k],
                rdests=_rdests(HOPS[k], False), queue_num=k,
            )

        for kc in range(2):
            nc.sync.dma_start(W_sb[:, kc * D:(kc + 1) * D], W[kc * P:(kc + 1) * P, :])
        nc.vector.tensor_copy(Wb_sb[:], W_sb[:])

        # ---- phase A: load shard, row norms, colsum(xn) partial ----
        from concourse.bass_types import AP as _AP
        for i in range(T):
            xs = x_all[:, i * D:(i + 1) * D]
            nc.sync.dma_start(xs, x[i * P:(i + 1) * P, :])
            scr = wp.tile([P, D], F32, tag="scr", name=f"scr{i}")
            nc.scalar.activation(scr[:], xs, AF.Square, accum_out=ss[:, i:i + 1])
            nc.vector.tensor_copy(xb_all[:, i * D:(i + 1) * D], xs)
        nc.scalar.activation(nrm[:], ss[:], AF.Sqrt)
        nc.vector.reciprocal(invn[:], nrm[:])
        invn_b = pp.tile([P, T], BF16)
        nc.vector.tensor_copy(invn_b[:], invn[:])

        # t partial in column layout: t_col[p, c] = sum_i (x_i chunk c)^T invn_i
        # (bf16 operands: one-pass matmuls, ~3x faster than fp32)
        psum_tc = psw.tile([P, D], F32, tag="pw", name="psum_tc")
        for c in range(2):
            for i in range(T):
                nc.tensor.matmul(
                    psum_tc[:, c:c + 1],
                    lhsT=xb_all[:, i * D + c * P:i * D + (c + 1) * P],
                    rhs=invn_b[:, i:i + 1],
                    start=(i == 0), stop=(i == T - 1),
                )
        nc.vector.tensor_copy(t_col[:, 0:2], psum_tc[:, 0:2])

        # x.T via PE transposes (independent work that overlaps the exchange)
        for i in range(T):
            for c in range(2):
                pt = psw.tile([P, P], F32, tag="pw", name=f"pt{i}_{c}")
                nc.tensor.transpose(
                    pt[:], x_all[:, i * D + c * P: i * D + (c + 1) * P], ident[:]
                )
                nc.vector.tensor_copy(xT_all[:, c * NL + i * P: c * NL + (i + 1) * P], pt[:])

        # y = x @ W per tile, also during the exchange window.  The zw
        # partial is then g^T @ y directly — no zT staging roundtrip.
        y_all = pp.tile([P, T * D], BF16)
        for i in range(T):
            py = psw.tile([P, D], F32, tag="pw", name=f"py{i}")
            for c in range(2):
                nc.tensor.matmul(
                    py[:], lhsT=xT_all[:, c * NL + i * P:c * NL + (i + 1) * P],
                    rhs=Wb_sb[:, c * D:(c + 1) * D],
                    start=(c == 0), stop=(c == 1),
                )
            nc.scalar.mul(y_all[:, i * D:(i + 1) * D], py[:], 1.0)

        # ---- t recursive doubling.  No entry barrier is needed:
        # target_bir_lowering is off so there is no per-kernel sem clear —
        # sems are zeroed at NEFF load and remote increments persist even if
        # a peer has not started executing yet.  Each trigger's
        # signals_writable gives it a WAW edge after the t_col producer (the
        # preps predate the producer, so the deferred-RAW edge never forms),
        # orders the hop's consumer add after it, and pins the next hop's
        # prep behind it in the queue-0 FIFO.
        add_t = []
        for k in range(3):
            # signals: WAW after the ta[k] producer, before the thr[k]
            # consumer, and pinning queue-k's zw prep (writes zhr[k])
            # behind this trigger in the FIFO.
            nc.gpsimd.trigger_dma(
                count=None, queue_num=k,
                signals_writable=(ta[k][:], thr[k][:], zhr[k][:]),
            )
            # Double-buffered: the sum lands in a fresh tile, so the hop's
            # in-flight send never races the accumulation (no local-sem wait).
            add_t.append(nc.vector.tensor_add(ta[k + 1][:], ta[k][:], thr[k][:]))

        # zw-hop send descriptors, queue k (pinned behind t-trigger k).
        # The running sum stays bf16; only the wire format is fp8 (e4m3),
        # which halves the per-descriptor bytes — the per-hop wall is
        # 64 descriptors per lane whose cost scales with payload bytes.
        for k, dq in enumerate(HOPS):
            nc.gpsimd.remote_dma_broadcast(
                zhr[k][:], zs8[k][:], rsem_z[k], lsem[k],
                rdests=_rdests(dq, True), queue_num=k,
            )

        # ---- phase B: degrees, f, g = f*x, zT partial, zw partial ----
        # Per-row dot products s = x . t directly on the PE using the
        # already-transposed x: psum_s[:, i] = sum_c xT(c,i)^T @ t_col_b[c]
        # — no 128-partition broadcast of t and no big multiply+reduce.
        tcol_b = pp.tile([P, 2], BF16)
        nc.vector.tensor_copy(tcol_b[:], ta[3][:, 0:2])
        psum_s = psw.tile([P, D], F32, tag="pw", name="psum_s")
        for i in range(T):
            for c in range(2):
                nc.tensor.matmul(
                    psum_s[:, i:i + 1],
                    lhsT=xT_all[:, c * NL + i * P:c * NL + (i + 1) * P],
                    rhs=tcol_b[:, c:c + 1],
                    start=(c == 0), stop=(c == 1),
                )
        nc.vector.tensor_copy(stl[:], psum_s[:, 0:T])
        # f = d * invn = rsqrt(s)/nrm with s = stl*invn  =>  f = rsqrt(stl*nrm)
        nc.vector.tensor_mul(s_t[:], stl[:], nrm[:])
        nc.scalar.activation(sq_s[:], s_t[:], AF.Sqrt)
        nc.vector.reciprocal(f_t[:], sq_s[:])
        # Split the per-row scaling g = f*x across ACT and DVE so neither
        # engine serializes all eight tiles.
        for i in range(T):
            gs = g_all[:, i * D:(i + 1) * D]
            xs = x_all[:, i * D:(i + 1) * D]
            if i % 2 == 0:
                nc.scalar.mul(gs, xs, f_t[:, i:i + 1])
            else:
                nc.vector.tensor_scalar_mul(gs, xs, f_t[:, i:i + 1])

        # zw partial = (g^T y) chunk m, accumulated over the row tiles.
        psum_zw0 = psp.tile([P, D], F32, name="pzw0")
        psum_zw1 = psp.tile([P, D], F32, name="pzw1")
        for i in range(T):
            for m, pz in ((0, psum_zw0), (1, psum_zw1)):
                nc.tensor.matmul(
                    pz[:], lhsT=g_all[:, i * D + m * P: i * D + (m + 1) * P],
                    rhs=y_all[:, i * D:(i + 1) * D],
                    start=(i == 0), stop=(i == T - 1),
                )
        for m, pz in ((0, psum_zw0), (1, psum_zw1)):
            nc.vector.tensor_copy(zw_loc[:, m * D:(m + 1) * D], pz[:])

        # ---- zw recursive doubling (hops on queues 1-3) ----
        add_z = []
        for k in range(3):
            nc.vector.tensor_copy(zs8[k][:], za[k][:])
            nc.gpsimd.trigger_dma(count=None, queue_num=k,
                                  signals_writable=(zs8[k][:], zhr[k][:]))
            add_z.append(nc.vector.tensor_add(za[k + 1][:], za[k][:], zhr[k][:]))

        # ---- phase C: out = f * (x @ zw) ----
        for i in range(T):
            po = psw.tile([P, D], F32, tag="pw", name=f"po{i}")
            for ka in range(2):
                nc.tensor.matmul(
                    po[:], lhsT=xT_all[:, ka * NL + i * P: ka * NL + (i + 1) * P],
                    rhs=za[3][:, ka * D:(ka + 1) * D],
                    start=(ka == 0), stop=(ka == 1),
                )
            o_sb = wp.tile([P, D], F32, tag="osb", name=f"osb{i}")
            nc.scalar.mul(o_sb[:], po[:], f_t[:, i:i + 1])
            nc.sync.dma_start(out[i * P:(i + 1) * P, :], o_sb[:])

    return {"add_t": add_t, "add_z": add_z,
            "rsem_t": rsem_t, "rsem_z": rsem_z, "lsem": lsem}


def _build():
    nc = bacc.Bacc("TRN2", target_bir_lowering=False, debug=False, num_devices=R,
                   num_swdge_queues=4)
    x = nc.dram_tensor("x", [NL, D], F32, kind="ExternalInput")
    W = nc.dram_tensor("W", [D, D], F32, kind="ExternalInput")
    out = nc.dram_tensor("out", [NL, D], F32, kind="ExternalOutput")
    with tile.TileContext(nc) as tc:
        h = _program(tc, x.ap() if hasattr(x, "ap") else x, W.ap() if hasattr(W, "ap") else W, out.ap() if hasattr(out, "ap") else out)
    # Attach the cross-core waits after scheduling (the schedule-time
    # single-core sim cannot model peer sem increments, and added waits
    # only delay — they cannot invalidate the schedule).  Each hop's add
    # waits for the partner's payload (+2 on the hop's remote sem) and for
    # this core's own send of the hop to drain (+16 on the queue's local
    # sem) before overwriting the send buffer.  compile() splits
    # multi-wait instructions into event semaphores automatically.
    for k in range(3):
        h["add_t"][k].wait_op(h["rsem_t"][k], _rinc(HOPS[k], False), "sem-ge", check=False)
        h["add_z"][k].wait_op(h["rsem_z"][k], _rinc(HOPS[k], True), "sem-ge", check=False)
    nc.finalize()
    return nc


def _run(inputs, trace=False):
    if "nc" not in _cache:
        _cache["nc"] = _build()
    nc = _cache["nc"]
    x = np.ascontiguousarray(inputs["x"], dtype=np.float32)
    W = np.ascontiguousarray(inputs["W"], dtype=np.float32)
    in_maps = [{"x": x[r * NL:(r + 1) * NL], "W": W} for r in range(R)]
    res = bass_utils.run_bass_kernel_spmd(
        nc, in_maps, core_ids=list(range(R)), trace=trace,
    )
    out = np.concatenate([res.results[r]["out"] for r in range(R)], axis=0)
    return out, res


def kernel(**inputs) -> np.ndarray:
    out, _ = _run(inputs, trace=False)
    return out


# revision 76
# speedup vs baseline: 1.2178x; 1.0216x over previous
"""Distributed Trainium2 kernel for the dense-graph GNN layer.

Math: with xn = x/||x|| (rows), G = xn@xn.T, d = rsqrt(G@1),
out = (diag(d) G diag(d) x) W.  The N x N Gram matrix is never needed:
  G @ 1        = xn @ t,            t = colsum(xn)            [D]
  diag(d) G diag(d) x = f * (x @ z),  z = x.T @ diag(f) @ x   [D, D]
  f_i = d_i / ||x_i||   (combines both scalings; z is symmetric)
  out = f * (x @ (z @ W)) = f * (x @ (x.T @ (f*x) @ W))
Each core processes its 1024-row shard; the only cross-core traffic is
(1) a reduction of the [D] colsum partial and (2) a reduction of the
[D, D] (z @ W) partial.  Both reductions run as recursive-doubling
exchanges over direct peer-to-peer SBUF remote DMAs (XOR partners
4, 2, 1 — cross-die first, so the launch-skew wait overlaps it), which
avoids the ~70us ncfw collective bringup entirely.  Hop accumulation is
double-buffered so an in-flight send never races the next sum.  A
dangling 1-byte AllGather marks the NEFF as collective so the runtime
gang-launches the 8 cores (without it, dispatch is staggered by
milliseconds).  The cross-core arrival waits are attached to the hop
adds after Tile scheduling — the schedule-time single-core sim cannot
model peer sem increments, and target_bir_lowering=False means sems are
zeroed at NEFF load (not per execution), so remote increments persist
even if a peer has not started executing yet and no entry barrier is
needed.
"""

import os
import sys

import numpy as np

for _p in ("/opt/trn_rl_repo", "/root/.axon_site/_ro/trn_rl_repo"):
    if os.path.isdir(_p) and _p not in sys.path:
        sys.path.insert(0, _p)

import concourse.bacc as bacc
import concourse.mybir as mybir
import concourse.tile as tile
import concourse.masks as masks
from concourse import bass_utils

R = 8                 # cores
N, D = 8192, 256
NL = N // R           # 1024 rows per core
P = 128
T = NL // P           # 8 row tiles per core
F32 = mybir.dt.float32
BF16 = mybir.dt.bfloat16
AF = mybir.ActivationFunctionType
ALU = mybir.AluOpType

TSLOT = 8             # t exchange payload width (f32 cols) = 32 B/partition
ZSLOT = 2 * D         # zw exchange payload width (bf16 cols) = 1 KB/partition
HOPS = (4, 2, 1)      # recursive-doubling XOR distances (cross-die first)

_cache = {}


def _rdests(dq, big):
    """8-slot dest list.  Every real slot ships a full 128-partition copy
    on its two lanes (64 descriptors/lane — invariant), so slots trade
    replication wire against dummy-lane trickle.  Small payloads (t) use
    all-same-dest slots wherever allowed: no dummy descriptors, and the
    replicated wire is negligible.  The 1 KB/partition zw payload keeps
    4 copies (halves the wire).  Cross-die dests (dq=4) may only occupy
    the D2D-capable slots 4-7.  Receiver sem increments: 2 per real slot."""
    if dq == 4:
        return [None, None, None, None, (0, dq), (0, dq), (0, dq), (0, dq)]
    if big:
        return [(0, dq)] * 4 + [None] * 4
    return [(0, dq)] * 8


def _rinc(dq, big):
    return 8 if (dq == 4 or big) else 16


def _program(tc, x, W, out):
    nc = tc.nc
    # Per-hop arrival sems (a shared counter would be ambiguous: a fast
    # partner's hop-2 arrival must not satisfy a hop-1 wait).
    rsem_t = [nc.alloc_semaphore(f"rsem_t{k}") for k in range(3)]
    rsem_z = [nc.alloc_semaphore(f"rsem_z{k}") for k in range(3)]
    # Local (send-drained) sems, one per SWDGE queue.
    lsem = [nc.alloc_semaphore(f"lsem_q{q}") for q in range(4)]
    with (
        tc.tile_pool(name="persist", bufs=1) as pp,
        tc.tile_pool(name="work", bufs=3) as wp,
        tc.tile_pool(name="psum", bufs=1, space="PSUM") as psp,
        tc.tile_pool(name="psumw", bufs=4, space="PSUM") as psw,
        tc.tile_pool(name="dram", bufs=1, space="DRAM") as dp,
    ):
        # Dangling 1-byte AllGather: marks the NEFF as collective so the
        # runtime gang-launches the 8 cores; nothing waits on it.
        cc_in = dp.tile([1, 1], F32)
        cc_out = dp.tile([R, 1], F32)
        nc.gpsimd.collective_compute(
            "AllGather", ALU.bypass, replica_groups=[list(range(R))],
            ins=[cc_in.opt()], outs=[cc_out.opt()],
        )

        ident = pp.tile([P, P], F32)
        masks.make_identity(nc, ident[:])
        x_all = pp.tile([P, T * D], F32)      # row tile i at [:, i*D:(i+1)*D]
        xb_all = pp.tile([P, T * D], BF16)    # bf16 copy of x
        g_all = pp.tile([P, T * D], BF16)     # f * x (bf16)
        xT_all = pp.tile([P, 2 * NL], BF16)   # x.T chunk c at [:, c*NL + i*P]
        W_sb = pp.tile([P, 2 * D], F32)       # W k-chunk kc at [:, kc*D]
        Wb_sb = pp.tile([P, 2 * D], BF16)

        ss = pp.tile([P, T], F32)
        invn = pp.tile([P, T], F32)
        nrm = pp.tile([P, T], F32)
        stl = pp.tile([P, T], F32)
        s_t = pp.tile([P, T], F32)
        sq_s = pp.tile([P, T], F32)
        dd = pp.tile([P, T], F32)
        f_t = pp.tile([P, T], F32)


        # Exchange buffers.  t_col / zw_loc accumulate in place; thr/zhr
        # receive the partner's running sum each hop.
        t_col = pp.tile([P, TSLOT], F32)      # my colsum partial, cols 0-1
        thr = [pp.tile([P, TSLOT], F32, name=f"thr{k}") for k in range(3)]
        ta = [t_col] + [pp.tile([P, TSLOT], F32, name=f"ta{k}") for k in range(3)]
        zw_loc = pp.tile([P, ZSLOT], BF16)    # my (z @ W) partial
        F8 = mybir.dt.float8e4
        zs8 = [pp.tile([P, ZSLOT], F8, name=f"zs8_{k}") for k in range(3)]
        zhr = [pp.tile([P, ZSLOT], F8, name=f"zhr{k}") for k in range(3)]
        za = [zw_loc] + [pp.tile([P, ZSLOT], BF16, name=f"za{k}") for k in range(3)]

        nc.gpsimd.memset(t_col[:], 0.0)

        # Hop k of both reductions lives on SWDGE queue k: a queue never
        # hosts two consecutive hops, so one hop's background dummy-lane
        # trickle cannot delay the next hop's descriptors.  The t-hop
        # preps are emitted here (desc-gen only, reads deferred).
        for k in range(3):
            nc.gpsimd.remote_dma_broadcast(
                thr[k][:], ta[k][:], rsem_t[k], lsem[k],
                rdests=_rdests(HOPS[k], False), queue_num=k,
            )

        for kc in range(2):
            nc.sync.dma_start(W_sb[:, kc * D:(kc + 1) * D], W[kc * P:(kc + 1) * P, :])
        nc.vector.tensor_copy(Wb_sb[:], W_sb[:])

        # ---- phase A: load shard, row norms, colsum(xn) partial ----
        from concourse.bass_types import AP as _AP
        invn_b = pp.tile([P, T], BF16)
        H = T // 2
        for h in range(2):
            for i in range(h * H, (h + 1) * H):
                xs = x_all[:, i * D:(i + 1) * D]
                nc.sync.dma_start(xs, x[i * P:(i + 1) * P, :])
                scr = wp.tile([P, D], F32, tag="scr", name=f"scr{i}")
                nc.scalar.activation(scr[:], xs, AF.Square, accum_out=ss[:, i:i + 1])
                nc.vector.tensor_copy(xb_all[:, i * D:(i + 1) * D], xs)
            # Per-half norms so the first colsum matmuls start while the
            # second half's squares are still running.
            hs = slice(h * H, (h + 1) * H)
            nc.scalar.activation(nrm[:, hs], ss[:, hs], AF.Sqrt)
            nc.vector.reciprocal(invn[:, hs], nrm[:, hs])
            nc.vector.tensor_copy(invn_b[:, hs], invn[:, hs])

        # t partial in column layout: t_col[p, c] = sum_i (x_i chunk c)^T invn_i
        # (bf16 operands: one-pass matmuls; i-major order so the first
        # half's matmuls run as soon as its invn half is ready)
        psum_tc0 = psw.tile([P, D], F32, tag="pw", name="psum_tc0")
        psum_tc1 = psw.tile([P, D], F32, tag="pw", name="psum_tc1")
        for i in range(T):
            for c, ptc in ((0, psum_tc0), (1, psum_tc1)):
                nc.tensor.matmul(
                    ptc[:, 0:1],
                    lhsT=xb_all[:, i * D + c * P:i * D + (c + 1) * P],
                    rhs=invn_b[:, i:i + 1],
                    start=(i == 0), stop=(i == T - 1),
                )
        nc.vector.tensor_copy(t_col[:, 0:1], psum_tc0[:, 0:1])
        nc.vector.tensor_copy(t_col[:, 1:2], psum_tc1[:, 0:1])

        # x.T via PE transposes (independent work that overlaps the exchange)
        for i in range(T):
            for c in range(2):
                pt = psw.tile([P, P], F32, tag="pw", name=f"pt{i}_{c}")
                nc.tensor.transpose(
                    pt[:], x_all[:, i * D + c * P: i * D + (c + 1) * P], ident[:]
                )
                nc.vector.tensor_copy(xT_all[:, c * NL + i * P: c * NL + (i + 1) * P], pt[:])

        # y = x @ W per tile, also during the exchange window.  The zw
        # partial is then g^T @ y directly — no zT staging roundtrip.
        y_all = pp.tile([P, T * D], BF16)
        for i in range(T):
            py = psw.tile([P, D], F32, tag="pw", name=f"py{i}")
            for c in range(2):
                nc.tensor.matmul(
                    py[:], lhsT=xT_all[:, c * NL + i * P:c * NL + (i + 1) * P],
                    rhs=Wb_sb[:, c * D:(c + 1) * D],
                    start=(c == 0), stop=(c == 1),
                )
            nc.scalar.mul(y_all[:, i * D:(i + 1) * D], py[:], 1.0)

        # ---- t recursive doubling.  No entry barrier is needed:
        # target_bir_lowering is off so there is no per-kernel sem clear —
        # sems are zeroed at NEFF load and remote increments persist even if
        # a peer has not started executing yet.  Each trigger's
        # signals_writable gives it a WAW edge after the t_col producer (the
        # preps predate the producer, so the deferred-RAW edge never forms),
        # orders the hop's consumer add after it, and pins the next hop's
        # prep behind it in the queue-0 FIFO.
        add_t = []
        for k in range(3):
            # signals: WAW after the ta[k] producer, before the thr[k]
            # consumer, and pinning queue-k's zw prep (writes zhr[k])
            # behind this trigger in the FIFO.
            nc.gpsimd.trigger_dma(
                count=None, queue_num=k,
                signals_writable=(ta[k][:], thr[k][:], zhr[k][:]),
            )
            # Double-buffered: the sum lands in a fresh tile, so the hop's
            # in-flight send never races the accumulation (no local-sem wait).
            add_t.append(nc.vector.tensor_add(ta[k + 1][:], ta[k][:], thr[k][:]))

        # zw-hop send descriptors, queue k (pinned behind t-trigger k).
        # The running sum stays bf16; only the wire format is fp8 (e4m3),
        # which halves the per-descriptor bytes — the per-hop wall is
        # 64 descriptors per lane whose cost scales with payload bytes.
        for k, dq in enumerate(HOPS):
            nc.gpsimd.remote_dma_broadcast(
                zhr[k][:], zs8[k][:], rsem_z[k], lsem[k],
                rdests=_rdests(dq, True), queue_num=k,
            )

        # ---- phase B: degrees, f, g = f*x, zT partial, zw partial ----
        # Per-row dot products s = x . t directly on the PE using the
        # already-transposed x: psum_s[:, i] = sum_c xT(c,i)^T @ t_col_b[c]
        # — no 128-partition broadcast of t and no big multiply+reduce.
        tcol_b = pp.tile([P, 2], BF16)
        nc.vector.tensor_copy(tcol_b[:], ta[3][:, 0:2])
        psum_s = psw.tile([P, D], F32, tag="pw", name="psum_s")
        for i in range(T):
            for c in range(2):
                nc.tensor.matmul(
                    psum_s[:, i:i + 1],
                    lhsT=xT_all[:, c * NL + i * P:c * NL + (i + 1) * P],
                    rhs=tcol_b[:, c:c + 1],
                    start=(c == 0), stop=(c == 1),
                )
        nc.vector.tensor_copy(stl[:], psum_s[:, 0:T])
        # f = d * invn = rsqrt(s)/nrm with s = stl*invn  =>  f = rsqrt(stl*nrm)
        nc.vector.tensor_mul(s_t[:], stl[:], nrm[:])
        nc.scalar.activation(sq_s[:], s_t[:], AF.Sqrt)
        nc.vector.reciprocal(f_t[:], sq_s[:])
        # Split the per-row scaling g = f*x across ACT and DVE so neither
        # engine serializes all eight tiles.
        for i in range(T):
            gs = g_all[:, i * D:(i + 1) * D]
            xs = x_all[:, i * D:(i + 1) * D]
            if i % 2 == 0:
                nc.scalar.mul(gs, xs, f_t[:, i:i + 1])
            else:
                nc.vector.tensor_scalar_mul(gs, xs, f_t[:, i:i + 1])

        # zw partial = (g^T y) chunk m, accumulated over the row tiles.
        psum_zw0 = psp.tile([P, D], F32, name="pzw0")
        psum_zw1 = psp.tile([P, D], F32, name="pzw1")
        for i in range(T):
            for m, pz in ((0, psum_zw0), (1, psum_zw1)):
                nc.tensor.matmul(
                    pz[:], lhsT=g_all[:, i * D + m * P: i * D + (m + 1) * P],
                    rhs=y_all[:, i * D:(i + 1) * D],
                    start=(i == 0), stop=(i == T - 1),
                )
        for m, pz in ((0, psum_zw0), (1, psum_zw1)):
            nc.vector.tensor_copy(zw_loc[:, m * D:(m + 1) * D], pz[:])

        # ---- zw recursive doubling (hops on queues 1-3) ----
        add_z = []
        for k in range(3):
            nc.vector.tensor_copy(zs8[k][:], za[k][:])
            nc.gpsimd.trigger_dma(count=None, queue_num=k,
                                  signals_writable=(zs8[k][:], zhr[k][:]))
            add_z.append(nc.vector.tensor_add(za[k + 1][:], za[k][:], zhr[k][:]))

        # ---- phase C: out = f * (x @ zw) ----
        for i in range(T):
            po = psw.tile([P, D], F32, tag="pw", name=f"po{i}")
            for ka in range(2):
                nc.tensor.matmul(
                    po[:], lhsT=xT_all[:, ka * NL + i * P: ka * NL + (i + 1) * P],
                    rhs=za[3][:, ka * D:(ka + 1) * D],
                    start=(ka == 0), stop=(ka == 1),
                )
            o_sb = wp.tile([P, D], F32, tag="osb", name=f"osb{i}")
            nc.scalar.mul(o_sb[:], po[:], f_t[:, i:i + 1])
            nc.sync.dma_start(out[i * P:(i + 1) * P, :], o_sb[:])

    return {"add_t": add_t, "add_z": add_z,
            "rsem_t": rsem_t, "rsem_z": rsem_z, "lsem": lsem}


def _build():
    nc = bacc.Bacc("TRN2", target_bir_lowering=False, debug=False, num_devices=R,
                   num_swdge_queues=4)
    x = nc.dram_tensor("x", [NL, D], F32, kind="ExternalInput")
    W = nc.dram_tensor("W", [D, D], F32, kind="ExternalInput")
    out = nc.dram_tensor("out", [NL, D], F32, kind="ExternalOutput")
    with tile.TileContext(nc) as tc:
        h = _program(tc, x.ap() if hasattr(x, "ap") else x, W.ap() if hasattr(W, "ap") else W, out.ap() if hasattr(out, "ap") else out)
    # Attach the cross-core waits after scheduling (the schedule-time
    # single-core sim cannot model peer sem increments, and added waits
    # only delay — they cannot invalidate the schedule).  Each hop's add
    # waits for the partner's payload (+2 on the hop's remote sem) and for
    # this core's own send of the hop to drain (+16 on the queue's local
    # sem) before overwriting the send buffer.  compile() splits
    # multi-wait instructions into event semaphores automatically.
    for k in range(3):
        h["add_t"][k].wait_op(h["rsem_t"][k], _rinc(HOPS[k], False), "sem-ge", check=False)
        h["add_z"][k].wait_op(h["rsem_z"][k], _rinc(HOPS[k], True), "sem-ge", check=False)
    nc.finalize()
    return nc


def _run(inputs, trace=False):
    if "nc" not in _cache:
        _cache["nc"] = _build()
    nc = _cache["nc"]
    x = np.ascontiguousarray(inputs["x"], dtype=np.float32)
    W = np.ascontiguousarray(inputs["W"], dtype=np.float32)
    in_maps = [{"x": x[r * NL:(r + 1) * NL], "W": W} for r in range(R)]
    res = bass_utils.run_bass_kernel_spmd(
        nc, in_maps, core_ids=list(range(R)), trace=trace,
    )
    out = np.concatenate([res.results[r]["out"] for r in range(R)], axis=0)
    return out, res


def kernel(**inputs) -> np.ndarray:
    out, _ = _run(inputs, trace=False)
    return out
